# revision 3
# baseline (speedup 1.0000x reference)
"""Trainium2 Bass kernel for a causal single-head attention block (v6).

Problem: y = softmax(mask(Q K^T / sqrt(H))) V with
  x  [B=4, T=4096, C=1024] f32,  Wq/Wk/Wv [C, H=64] f32.

Sharding (8 NeuronCores, collective-free): one core PAIR per batch.
Both cores of a pair compute Q for ALL 8 q-tiles (512 rows each) of
their batch; the k-range is split by 256-row k-pairs: rank r owns
global k-pairs {p : p % 2 == r}.  Causal work is exactly balanced
(tile g needs pairs 0..2g+1 -> g+1 pairs per rank, one diagonal).
Each core emits UNNORMALIZED partial y plus a partial row-sum row
(ones-column trick in the PV stationary); the host adds the two
partials and divides.  No AllGather, no inter-core deps: the graph is
identical on all cores (SPMD); all rank-dependence is input DATA
(a block-permuted x layout and a mask sheet).

Per-core stream over 8 t-blocks (512 rows, block g == q-tile g):
 - x^T block DMA (1 MiB, sync queue), host-packed so the core's OWN
   k-pair rows are always the first 256 columns of the block.
 - K/V projection of the own pair: stationary [Wk|Wv] for the even
   128-chunk and [Wv|Wk] for the odd chunk, so K^T lands on SBUF
   partitions 0:64 (even) / 64:128 (odd) with same-partition copies.
 - V^T -> V t-layout via PE transposes (identity carries eye(64) in
   both partition halves so even/odd chunks transpose in place).
 - Q projection ([Wq|Wq] stationary duplicates Q^T to both partition
   halves so the two S^T matmuls of a pair run on disjoint PE row
   groups / PSUM banks concurrently).
 - Attention for tile g immediately (all its K/V is resident by
   causality): per local pair: 2 S^T matmuls -> exp on ACT (scale
   folded) -> (diagonal only) bf16 mask multiply on DVE -> 2 PV
   matmuls (width 65: ones column accumulates the row-sum for free)
   -> per-tile evac + DMA out.

PSUM budget (8 banks): S pipeline 2x[128,1024] (4) + y accumulator
2x[128,512] (2) + KV proj (1) + Q proj (1).
"""

import numpy as np
import ml_dtypes

import concourse.bass as bass
import concourse.bacc as bacc
import concourse.mybir as mybir
from concourse.tile import TileContext
from concourse.bass_utils import run_bass_kernel_spmd

BF16 = mybir.dt.bfloat16
F32 = mybir.dt.float32
bf16 = ml_dtypes.bfloat16

B, T, C, H = 4, 4096, 1024, 64
N_CORES = 8
NBLK = 8            # 512-row t-blocks == q-tiles per batch
QT = 512
SCALE = float(H) ** -0.5


def build_bass():
    nc = bacc.Bacc(
        "TRN2",
        target_bir_lowering=False,
        debug=False,
        enable_asserts=False,
        num_devices=N_CORES,
    )

    xT = nc.declare_dram_parameter("xT", [C, T], BF16, isOutput=False)
    wkv = nc.declare_dram_parameter("wkv", [C, 128], BF16, isOutput=False)
    wvk = nc.declare_dram_parameter("wvk", [C, 128], BF16, isOutput=False)
    wqq = nc.declare_dram_parameter("wqq", [C, 128], BF16, isOutput=False)
    mask = nc.declare_dram_parameter("mask", [128, 896], BF16, isOutput=False)
    ident = nc.declare_dram_parameter("ident", [128, H], BF16, isOutput=False)
    out = nc.declare_dram_parameter("out", [H + 1, T], BF16, isOutput=True)

    with TileContext(nc) as tc:
        with (
            tc.tile_pool(name="persist", bufs=1) as pp,
            tc.tile_pool(name="work", bufs=3) as wp,
            tc.tile_pool(name="swp", bufs=2, space="PSUM") as swp,
            tc.tile_pool(name="yp", bufs=1, space="PSUM") as yp,
            tc.tile_pool(name="kvp", bufs=1, space="PSUM") as kvp,
            tc.tile_pool(name="qp", bufs=1, space="PSUM") as qp,
            tc.tile_pool(name="vtp", bufs=1, space="PSUM") as vtp,
        ):
            # ---- persistent SBUF ----
            wkv_sb = pp.tile([128, 8, 128], BF16, tag="wkv")
            wvk_sb = pp.tile([128, 8, 128], BF16, tag="wvk")
            wqq_sb = pp.tile([128, 8, 128], BF16, tag="wqq")
            mask_sb = pp.tile([128, 896], BF16, tag="mask")
            id_sb = pp.tile([128, H], BF16, tag="ident")
            xb = [
                pp.tile([128, 8, QT], BF16, tag=f"xb{g}", name=f"xb_{g}")
                for g in range(NBLK)
            ]
            kT = [
                pp.tile([128, 128], BF16, tag=f"kT{g}", name=f"kT_{g}")
                for g in range(NBLK)
            ]
            # V t-layout chunk + ones col at 64; padded to 80 cols so each
            # tile start stays 32B-aligned for the xbar DMA-transpose.
            vaug = [
                pp.tile([128, 80], BF16, tag=f"va{c}", name=f"vaug_{c}")
                for c in range(2 * NBLK)
            ]
            qT2 = [
                pp.tile([128, QT], BF16, tag=f"q{g}", name=f"qT2_{g}")
                for g in range(NBLK)
            ]
            ones_sb = pp.tile([1, 8], F32, tag="ones")
            dume = pp.tile([1, 8], F32, tag="dume")
            dummy_w = pp.tile([128, QT], BF16, tag="dummyw")

            # ---- preamble ----
            nc.vector.memset(ones_sb[:], 1.0)
            nc.vector.memset(dummy_w[:], 0.5)
            for c in range(2 * NBLK):
                nc.vector.memset(vaug[c][:, 64:65], 1.0)
            # trigger the exp table-set load early on the ACT queue
            nc.scalar.activation(
                dume[:], ones_sb[:], mybir.ActivationFunctionType.Exp
            )
            # weights + mask on the gpsimd (SWDGE) queue; x stream on sync
            nc.gpsimd.dma_start(
                out=wkv_sb[:], in_=wkv[:].rearrange("(cc p) m -> p cc m", p=128)
            )
            nc.gpsimd.dma_start(
                out=wvk_sb[:], in_=wvk[:].rearrange("(cc p) m -> p cc m", p=128)
            )
            nc.gpsimd.dma_start(
                out=wqq_sb[:], in_=wqq[:].rearrange("(cc p) m -> p cc m", p=128)
            )
            nc.gpsimd.dma_start(out=mask_sb[:], in_=mask[:])
            nc.gpsimd.dma_start(out=id_sb[:], in_=ident[:])

            def load_x(g, nsplit=2):
                # cc-split DMAs so Q/KV accumulation starts on the first
                # chunk while the rest streams
                cw = 8 // nsplit
                for h in range(nsplit):
                    nc.sync.dma_start(
                        out=xb[g][:, cw * h : cw * (h + 1), :],
                        in_=xT[
                            h * cw * 128 : (h + 1) * cw * 128,
                            g * QT : (g + 1) * QT,
                        ].rearrange("(cc p) t -> p cc t", p=128),
                    )

            load_x(0, nsplit=1)
            load_x(1, nsplit=1)

            def warm(n):
                # scratch matmuls: keep PE activity up so the HAM clock
                # gate stays at 8/8 across DMA-bound stretches
                for wi in range(n):
                    wps = swp.tile([128, 1024], F32, tag="sw", name="warm")
                    nc.tensor.matmul(
                        wps[:, 0:QT],
                        dummy_w[:, 0:128],
                        dummy_w[:],
                        start=True,
                        stop=True,
                    )

            warm(10)

            # ---- main stream ----
            for g in range(NBLK):
                gsl = slice(g * QT, (g + 1) * QT)

                # Q projection (full block, Q^T duplicated to both halves)
                q = qp.tile([128, QT], F32, tag="q")
                for cc in range(8):
                    nc.tensor.matmul(
                        q[:],
                        wqq_sb[:, cc, :],
                        xb[g][:, cc, :],
                        start=(cc == 0),
                        stop=(cc == 7),
                    )
                nc.vector.tensor_copy(qT2[g][:], q[:])

                # K/V projection of the own pair (block-local cols 0:256)
                kv = kvp.tile([128, 256], F32, tag="kv")
                for half, wsb in ((0, wkv_sb), (1, wvk_sb)):
                    for cc in range(8):
                        nc.tensor.matmul(
                            kv[:, half * 128 : (half + 1) * 128],
                            wsb[:, cc, :],
                            xb[g][:, cc, half * 128 : (half + 1) * 128],
                            start=(cc == 0),
                            stop=(cc == 7),
                        )
                kvst = wp.tile([128, 256], BF16, tag="kvst")
                nc.vector.tensor_copy(kvst[:], kv[:])
                # K^T: even chunk on partitions 0:64, odd on 64:128
                nc.vector.tensor_copy(kT[g][0:64, :], kvst[0:64, 0:128])
                nc.vector.tensor_copy(kT[g][64:128, :], kvst[64:128, 128:256])
                # V^T -> V t-layout: PE transposes (even chunk sits on
                # partitions 64:128, odd chunk on 0:64)
                for half, psl in ((0, slice(64, 128)), (1, slice(0, 64))):
                    vt = vtp.tile([128, H], BF16, tag="vt")
                    nc.tensor.transpose(
                        vt[:],
                        kvst[psl, half * 128 : (half + 1) * 128],
                        id_sb[psl, :],
                    )
                    nc.vector.tensor_copy(vaug[2 * g + half][:, 0:64], vt[:])
                if g + 2 < NBLK:
                    load_x(g + 2, nsplit=1)

                # attention for tile g over local pairs 0..g, software-
                # pipelined and batched by two ops so PE tiling-mode
                # switches (row-tiled S vs full-array PV) amortize
                y = yp.tile([128, QT], F32, tag="y")

                def s_exp(lp):
                    diag = lp == g
                    w2 = 384 if diag else 512
                    sw = swp.tile([128, 1024], F32, tag="sw")
                    nc.tensor.matmul(
                        sw[:, 0:QT],
                        kT[lp][0:64, :],
                        qT2[g][0:64, :],
                        start=True,
                        stop=True,
                    )
                    nc.tensor.matmul(
                        sw[:, QT : QT + w2],
                        kT[lp][64:128, :],
                        qT2[g][64:128, QT - w2 : QT],
                        start=True,
                        stop=True,
                    )
                    pt = wp.tile([128, 1024], BF16, tag="pt")
                    nc.scalar.activation(
                        pt[:, 0 : QT + w2],
                        sw[:, 0 : QT + w2],
                        mybir.ActivationFunctionType.Exp,
                        scale=SCALE,
                    )
                    if diag:
                        nc.vector.tensor_mul(
                            pt[:, 0:896], pt[:, 0:896], mask_sb[:]
                        )
                    return pt, w2

                order = [g] + list(range(g))  # diagonal first

                def pv(lp, pt, w2):
                    first = lp == order[0]
                    last = lp == order[-1]
                    nc.tensor.matmul(
                        y[0 : H + 1, :],
                        vaug[2 * lp][:, 0:65],
                        pt[:, 0:QT],
                        start=first,
                        stop=False,
                        skip_group_check=True,
                    )
                    nc.tensor.matmul(
                        y[0 : H + 1, QT - w2 : QT],
                        vaug[2 * lp + 1][:, 0:65],
                        pt[:, QT : QT + w2],
                        start=False,
                        stop=last,
                        skip_group_check=True,
                    )

                prev = None
                for lp in order:
                    cur = (lp, *s_exp(lp))
                    if prev is not None:
                        pv(*prev)
                    prev = cur
                pv(*prev)
                ysb = wp.tile([H + 1, QT], BF16, tag="ysb")
                nc.vector.tensor_copy(ysb[:], y[0 : H + 1, :])
                nc.sync.dma_start(out=out[:, gsl], in_=ysb[:])

    nc.compile()
    return nc


_NC_CACHE = None


def _get_nc():
    global _NC_CACHE
    if _NC_CACHE is None:
        _NC_CACHE = build_bass()
    return _NC_CACHE


def _make_mask(r):
    """Mask sheet [128, 896] for the diagonal op, rank r."""
    p = np.arange(128)[:, None]
    m = np.zeros((128, 896), dtype=bf16)
    if r == 0:
        q = np.arange(512)[None, :]
        m[:, 0:512] = (p <= q).astype(bf16)
        j = np.arange(384)[None, :]
        m[:, 512:896] = (p <= j).astype(bf16)
    else:
        c = np.arange(256)[None, :]
        m[:, 0:256] = (p <= c).astype(bf16)
        j = np.arange(128)[None, :]
        m[:, 512:640] = (p <= j).astype(bf16)
    return m


def _make_in_maps(x, Wq, Wk, Wv):
    wkv = np.concatenate([Wk, Wv], axis=1).astype(bf16)
    wvk = np.concatenate([Wv, Wk], axis=1).astype(bf16)
    wqq = np.concatenate([Wq, Wq], axis=1).astype(bf16)
    ident = np.zeros((128, H), dtype=bf16)
    ident[0:64, :] = np.eye(H, dtype=bf16)
    ident[64:128, :] = np.eye(H, dtype=bf16)
    masks = [_make_mask(0), _make_mask(1)]
    in_maps = []
    for c in range(N_CORES):
        b, r = divmod(c, 2)
        xp = x[b]
        if r == 1:
            # swap the 256-row halves of each 512 block: own pair first
            xp = xp.reshape(8, 2, 256, C)[:, ::-1].reshape(T, C)
        xT_c = np.ascontiguousarray(xp.T).astype(bf16)
        in_maps.append(
            {
                "xT": xT_c,
                "wkv": wkv,
                "wvk": wvk,
                "wqq": wqq,
                "mask": masks[r],
                "ident": ident,
            }
        )
    return in_maps


def _assemble(results):
    y = np.empty((B, T, H), dtype=np.float32)
    for b in range(B):
        a = np.asarray(results[2 * b]["out"], dtype=np.float32)
        o = np.asarray(results[2 * b + 1]["out"], dtype=np.float32)
        # rank 1 columns are block-half permuted; undo before combining
        o = o.reshape(H + 1, 8, 2, 256)[:, :, ::-1].reshape(H + 1, T)
        num = a[0:H] + o[0:H]
        den = a[H] + o[H]
        y[b] = (num / den[None, :]).T
    return y


def run(x, Wq, Wk, Wv, trace=False):
    nc = _get_nc()
    in_maps = _make_in_maps(
        np.asarray(x, np.float32),
        np.asarray(Wq, np.float32),
        np.asarray(Wk, np.float32),
        np.asarray(Wv, np.float32),
    )
    res = run_bass_kernel_spmd(nc, in_maps, core_ids=list(range(N_CORES)), trace=trace)
    return _assemble(res.results), res


def kernel(x, Wq, Wk, Wv):
    y, _ = run(x, Wq, Wk, Wv)
    return y
